# revision 47
# baseline (speedup 1.0000x reference)
"""Trainium2 Bass kernel for nn_BodyKinematics (batched tree forward kinematics).

Contract: kernel(**inputs) takes the FULL unsharded inputs as numpy arrays and
returns the FULL output (B, N, 4, 4) float32.  Internally the batch dim is
sharded across 8 NeuronCores (pure data parallelism); the tiny per-edge
parameters are replicated.

Math (matches the jax reference):
  theta = tanh(log_angles) * scale                     # (B, 3E), offset == 0
  per edge e: r = Rx(th_x) @ Ry(th_y) @ Rz(th_z)       # axes are e_x, e_y, e_z
  local_e  = [r | 0; 0 1] @ tip_to_base[e]             # affine 3x4 is enough
  tree: W_0 = I, W_n = W_parent(n) @ local_{n-1}       # parent(n) = (n-1)//2
  output: W as 4x4 with constant bottom row (0,0,0,1)

Device design (per core, 512 batch rows = 4 subtiles x 128 partitions):
  partitions = batch-within-subtile; free dim = per-edge structure with the
  edge/node index INNERMOST; fp16 compute everywhere so every TensorTensor
  hits the DVE 2x_1p fast mode (2-byte packed operands).  All 4 subtiles are
  merged into single wide instructions.

  Engine plan: DVE does BC edges [0,EB) plus the whole tree; GpSimd (slow at
  TensorTensor) independently does BC edges [EB,E) in pipelined e-chunks so
  its locals arrive progressively for the deep tree level; ACT does
  tanh/sin/|t|/cos, the parent->children replication copies for big tree
  levels, and the fp16->fp32 shuffle into the node-major fp32 output tile.
  Output DMAs are issued per node-range as soon as their shuffle lands.

  A single activation table (silu_and_others: tanh+sin+abs+copy) serves all
  ACT ops so no per-iteration table reloads happen.
"""

import os
import sys

for _p in ("/opt/trn_rl_repo",):
    if _p not in sys.path and os.path.isdir(_p):
        sys.path.insert(0, _p)

import numpy as np

B, E, N = 4096, 255, 256
J = 3 * E           # 765 angles
NCORE, P, NSUB = 8, 128, 4
BPC = P * NSUB      # 512 batch rows per core
PI = float(np.pi)

# BC edge-range split (tuning knobs)
EA = 62                           # chunk A: all-DVE, feeds small tree levels
EB = 250                          # DVE does [EA, EB); Pool does [EB, E)
POOL_CHUNKS = [250, 255]            # Pool pipeline chunk boundaries
# NOTE: GpSimd fp16 TensorTensor is ~1.7x slower on HW than the cost model,
# and concurrent Pool BC interferes with DVE non-monotonically (measured:
# Pool-95-edges 129us, Pool-67 146us, Pool-5 122us).  Best measured config
# keeps BC almost entirely on DVE.
JP = 3 * 256                      # axis-padded angle pitch (4B alignment)

_state: dict = {}


# --------------------------------------------------------------------------- #
# numpy fallback (exact float32 port of the reference) — used only if the
# inputs don't match the structure the device kernel was built for.
# --------------------------------------------------------------------------- #
def _np_skew(a):
    x, y, z = a[..., 0], a[..., 1], a[..., 2]
    zero = np.zeros_like(x)
    return np.stack([
        np.stack([zero, -z, y], -1),
        np.stack([z, zero, -x], -1),
        np.stack([-y, x, zero], -1)], -2)


def _np_fallback(log_angles, tip_to_base, rot_axes, rot_constraints):
    la = log_angles.astype(np.float32)
    b, e3 = la.shape
    e = e3 // 3
    n = e + 1
    theta = np.tanh(la) * rot_constraints[:, 0] + rot_constraints[:, 1]
    K = _np_skew(rot_axes.astype(np.float32))
    K2 = np.einsum('mij,mjk->mik', K, K).astype(np.float32)
    s = np.sin(theta)[..., None, None]
    c = (1.0 - np.cos(theta))[..., None, None]
    I3 = np.eye(3, dtype=np.float32)
    rots = (I3 + s * K + c * K2).reshape(b, e, 3, 3, 3).astype(np.float32)
    r = np.einsum('beij,bejk,bekl->beil', rots[:, :, 0], rots[:, :, 1],
                  rots[:, :, 2]).astype(np.float32)
    T = np.zeros((b, e, 4, 4), np.float32)
    T[..., :3, :3] = r
    T[..., 3, 3] = 1.0
    local = np.einsum('beij,ejk->beik', T,
                      tip_to_base.astype(np.float32)).astype(np.float32)
    worlds = np.zeros((b, n, 4, 4), np.float32)
    worlds[:, 0] = np.eye(4, dtype=np.float32)
    for i in range(1, n):
        par = (i - 1) // 2
        worlds[:, i] = (worlds[:, par] @ local[:, i - 1]).astype(np.float32)
    return worlds


# tree levels: node range [lo, hi), m = hi - lo nodes, edges [lo-1, hi-1)
SMALL_LEVELS = [(3, 7), (7, 15), (15, 31), (31, 63)]
D6 = (63, 127)
D7 = (127, 255)


def _patch_act_tables():
    """Restrict activation-table selection to silu_and_others (which holds
    tanh, sin, abs, copy) so a single LoadActFuncSet, hoisted out of the
    loop, serves every ACT op."""
    import concourse.hw_specs as hw_specs
    import concourse.bacc as bacc
    if getattr(bacc, "_act_tables_patched", False):
        return
    orig = hw_specs.get_activation_tables

    def patched(arch):
        tabs = orig(arch)
        keep = None
        for name, funcs in tabs.items():
            import concourse.mybir as mybir
            AFT = mybir.ActivationFunctionType
            if (AFT.Tanh in funcs and AFT.Sin in funcs and AFT.Abs in funcs
                    and AFT.Copy in funcs):
                keep = name
                break
        if keep is None:
            return tabs
        return {name: (funcs if name == keep else frozenset())
                for name, funcs in tabs.items()}

    bacc.get_activation_tables = patched
    bacc._act_tables_patched = True


# --------------------------------------------------------------------------- #
# device kernel build
# --------------------------------------------------------------------------- #
def _build_nc(general_constraints: bool, sc_const: float, of_const: float,
              loop_n: int = 1):
    assert not general_constraints, "fast path requires uniform constraints"
    import concourse.bacc as bacc
    import concourse.mybir as mybir
    from concourse.tile import TileContext
    import concourse.bass as bass
    from contextlib import ExitStack

    f32 = mybir.dt.float32
    f16 = mybir.dt.float16
    i32 = mybir.dt.int32
    Alu = mybir.AluOpType
    AFT = mybir.ActivationFunctionType

    scv = float(sc_const)
    assert float(of_const) == 0.0, "fast path assumes zero offset"

    _patch_act_tables()
    nc = bacc.Bacc("TRN2", target_bir_lowering=False, debug=False)

    la_d = nc.dram_tensor("la", [BPC, J], f32, kind="ExternalInput")
    # tip prepared host-side as (k, l, e-pad256) fp16, flat [1, 12*256]
    tip_d = nc.dram_tensor("tip", [1, 12 * 256], f16, kind="ExternalInput")
    out_d = nc.dram_tensor("out", [BPC, N * 16], f32, kind="ExternalOutput")

    with TileContext(nc) as tc:
        with tc.tile_pool(name="main", bufs=1) as pool, \
             tc.tile_pool(name="ps", bufs=1, space="PSUM") as psp, \
             ExitStack() as _loop_ctx:

            ED = EB              # DVE locals/scratch cover edges [0, EB)
            EP = E - EB          # Pool covers [EB, E)
            # angle tiles use a 256-padded axis pitch so every fp16
            # operand lands 4B-aligned (required for the DVE 2x_1p mode)
            la_t = pool.tile([P, NSUB * JP], f32)   # raw angles; reused as |t|
            th_t = pool.tile([P, NSUB * JP], f32)   # theta, (s, a, e) order
            tip_t = pool.tile([P, 12 * 256], f16)   # (k, l, e-padded)
            # NOTE: tiles shared by two engines must live in SBUF — the tile
            # scheduler serializes ALL cross-engine PSUM accesses (even
            # read-read), which would chain DVE behind every Pool op.
            sin_t = pool.tile([P, NSUB * JP], f16)  # (s, a, e-padded)
            cos_t = pool.tile([P, NSUB * JP], f16)
            # per-engine locals + scratch (separate tiles so the dep tracker
            # never serializes DVE behind Pool); Pool locals are per-chunk
            # tiles so the deep-level reads depend only on their own chunk.
            locd_t = pool.tile([P, NSUB * 12 * ED], f16)   # (s, k, l, e<EB)
            locp_t = {}
            locp_w = {}
            for c0, c1 in zip(POOL_CHUNKS[:-1], POOL_CHUNKS[1:]):
                cw = (c1 - c0 + 1) & ~1          # even-padded width
                locp_w[c0] = cw
                locp_t[c0] = pool.tile([P, NSUB * 12 * cw], f16,
                                       name=f"locp{c0}")
            # DVE scratch in PSUM (DVE-exclusive, so no cross-engine edges);
            # q2 aliases r0 (r0 is dead once the y-stage q2 mul runs).
            r0d_t = pool.tile([P, NSUB * 4 * ED], f16)
            r1d_t = pool.tile([P, NSUB * 4 * ED], f16)
            tAd_t = pool.tile([P, NSUB * 4 * ED], f16)
            EPW = (EP + 1) & ~1              # even-padded Pool width
            r0p_t = pool.tile([P, NSUB * 4 * EPW], f16)
            r1p_t = pool.tile([P, NSUB * 4 * EPW], f16)
            tAp_t = pool.tile([P, NSUB * 4 * EPW], f16)
            tBp_t = pool.tile([P, NSUB * 4 * EPW], f16)
            w_t = {}
            for (lo, hi) in SMALL_LEVELS + [D6]:
                m = hi - lo
                w_t[lo] = pool.tile([P, NSUB * 12 * m], f16, name=f"w{lo}")
            d7_bounds = sorted(set(
                [0, min(EB - 126, 128)] +
                [min(c - 126, 128) for c in POOL_CHUNKS[1:]]))
            i_ = 0                      # subdivide any chunk wider than 64
            while i_ < len(d7_bounds) - 1:
                a_, b_ = d7_bounds[i_], d7_bounds[i_ + 1]
                if b_ - a_ > 64:
                    d7_bounds.insert(i_ + 1, a_ + (b_ - a_ + 1) // 2 // 2 * 2)
                else:
                    i_ += 1
            w7c_t = {}
            for n0, n1 in zip(d7_bounds[:-1], d7_bounds[1:]):
                w7c_t[n0] = pool.tile([P, NSUB * 12 * (n1 - n0)], f16,
                                      name=f"w7c{n0}")
            w8_t = pool.tile([P, NSUB * 12], f16)        # node 255
            tmp_p8 = pool.tile([P, NSUB * 12], f16)      # node-255 scratch
            tmp_t = pool.tile([P, NSUB * 12 * 64], f16)  # chunk mul scratch
            repa_t = pool.tile([P, NSUB * 12 * 64], f16)
            repb_t = pool.tile([P, NSUB * 12 * 128], f16)
            # per-DMA-chunk output tiles (separate so a shuffle never WARs
            # against the previous chunk's DMA read)
            out_chunks = [(0, 31), (31, 63), (63, 127)] + \
                [(127 + a, 127 + b) for a, b in
                 zip(d7_bounds[:-2], d7_bounds[1:-1])] + \
                [(127 + d7_bounds[-2], 256)]
            out_ts = {}
            for (nlo, nhi) in out_chunks:
                out_ts[nlo] = pool.tile([P, NSUB * (nhi - nlo) * 16], f32,
                                        name=f"out{nlo}")
            hpi_t = pool.tile([P, 1], f32)

            def out_ap(node0, cnt, sub_off, dims_tail):
                # AP into the out-chunk tile holding [node0, node0+cnt):
                # dims: [partition] + [[chunk-sub-stride, NSUB]] + dims_tail
                for (nlo, nhi) in out_chunks:
                    if nlo <= node0 and node0 + cnt <= nhi:
                        cw = (nhi - nlo) * 16
                        return ap(out_ts[nlo],
                                  (node0 - nlo) * 16 + sub_off,
                                  [[cw, NSUB]] + dims_tail)
                raise AssertionError((node0, cnt))

            def ap(tile, off, dims):
                a0 = tile[:]
                return bass.AP(a0.tensor, a0.offset + off,
                               [list(a0.ap[0])] + [list(d) for d in dims])

            # ---- one-time init: pi/2 bias + constant parts of out tiles ----
            nc.gpsimd.memset(hpi_t[:], PI / 2.0)
            for (nlo, nhi) in out_chunks:
                cn = nhi - nlo
                ot = out_ts[nlo]
                nc.gpsimd.memset(ap(ot, 12, [[16, NSUB * cn], [1, 3]]), 0.0)
                nc.gpsimd.memset(ap(ot, 15, [[16, NSUB * cn]]), 1.0)
            for s in range(NSUB):
                nc.gpsimd.memset(ap(out_ts[0], s * 31 * 16,
                                    [[4, 3], [1, 4]]), 0.0)
                nc.gpsimd.memset(ap(out_ts[0], s * 31 * 16, [[5, 3]]), 1.0)

            if loop_n > 1:
                _loop_ctx.enter_context(tc.For_i(0, loop_n, 1))

            # ---------------- input DMAs ----------------
            la_v = la_d[:].rearrange("(s p) j -> p s j", p=P)   # [128, 4, 765]
            for s in range(NSUB):
                nc.sync.dma_start(la_t[:, s * JP:s * JP + J], la_v[:, s])
            nc.sync.dma_start(tip_t[:],
                              bass.AP(tip_d, 0, [[0, P], [1, 12 * 256]]))

            # ---------------- angles: tanh, then per-axis sin/|t|/cos -------
            act = nc.scalar.activation
            # tanh with fused (e,a) -> (a,e) deinterleave, per sub so each
            # starts as soon as its input DMA lands
            for s in range(NSUB):
                act(ap(th_t, s * JP, [[256, 3], [1, 255]]),
                    ap(la_t, s * JP, [[1, 3], [3, 255]]),
                    AFT.Tanh)
            # per axis (z first so BC stage z can start earliest):
            #   sin_a = Sin(scv * th_a); ab_a = |th_a|  (into la_t, (a,e));
            #   cos_a = Sin(pi/2 - scv*ab_a)
            for a_ax in (2, 1, 0):
                tha = ap(th_t, a_ax * 256, [[JP, NSUB], [1, 255]])
                sina = ap(sin_t, a_ax * 256, [[JP, NSUB], [1, 255]])
                aba = ap(la_t, a_ax * 256, [[JP, NSUB], [1, 255]])
                cosa = ap(cos_t, a_ax * 256, [[JP, NSUB], [1, 255]])
                act(sina, tha, AFT.Sin, scale=scv)
                act(aba, tha, AFT.Abs)
                act(cosa, aba, AFT.Sin, bias=hpi_t[:], scale=-scv)

            # ---------------- BC: locals = Rx Ry Rz @ tip ----------------
            #   z: r0 = cz*T0 - sz*T1 ; r1 = sz*T0 + cz*T1
            #   y: L0 = cy*r0 + sy*T2 ; q2 = cy*T2 - sy*r0
            #   x: L1 = cx*r1 - sx*q2 ; L2 = sx*r1 + cx*q2
            def emit_bc(eng, e0, e1, loc_tile, lbase, lw, scr, sbase, sw):
                w = e1 - e0
                lo_ = e0 - lbase
                so = e0 - sbase

                def trig(t, a_ax):
                    return ap(t, a_ax * 256 + e0,
                              [[JP, NSUB], [0, 4], [1, w]])

                def tipr(k):
                    return ap(tip_t, k * 4 * 256 + e0,
                              [[0, NSUB], [256, 4], [1, w]])

                def sle(t):
                    return ap(t, so, [[4 * sw, NSUB], [sw, 4], [1, w]])

                def locr(k):
                    return ap(loc_tile, k * 4 * lw + lo_,
                              [[12 * lw, NSUB], [lw, 4], [1, w]])

                sx, sy, sz = (trig(sin_t, a) for a in range(3))
                cx, cy, cz = (trig(cos_t, a) for a in range(3))
                T0, T1, T2 = (tipr(k) for k in range(3))
                L0, L1, L2 = (locr(k) for k in range(3))
                r0, r1, tA = (sle(t) for t in scr[:3])
                tB = sle(scr[3]) if len(scr) > 3 else None
                q2 = r0        # alias: r0 dead after the q2 in-place mul
                stages = [
                    [(cz, T0, sz, T1, r0, Alu.subtract, True),
                     (sz, T0, cz, T1, r1, Alu.add, False)],
                    [(cy, r0, sy, T2, L0, Alu.add, False),
                     (cy, T2, sy, r0, q2, Alu.subtract, True)],
                    [(cx, r1, sx, q2, L1, Alu.subtract, True),
                     (sx, r1, cx, q2, L2, Alu.add, False)],
                ]
                tt = eng.tensor_tensor
                if tB is not None:
                    # per stage: both triples' muls first (two tA buffers),
                    # then the combines — keeps a 2-wide ready window so
                    # Pool never idles into a cross-chunk slip-in.
                    for (t1, t2) in stages:
                        (a1, b1, c1, d1, dst1, op1, f1) = t1
                        (a2, b2, c2, d2, dst2, op2, f2) = t2
                        tt(tA, a1, b1, Alu.mult)
                        tt(dst1, c1, d1, Alu.mult)
                        tt(tB, a2, b2, Alu.mult)
                        tt(dst2, c2, d2, Alu.mult)
                        tt(dst1, tA, dst1, op1) if f1 else \
                            tt(dst1, dst1, tA, op1)
                        tt(dst2, tB, dst2, op2) if f2 else \
                            tt(dst2, dst2, tB, op2)
                else:
                    for (t1, t2) in stages:
                        for (a, b, c, d, dst, op, f) in (t1, t2):
                            tt(tA, a, b, Alu.mult)
                            tt(dst, c, d, Alu.mult)
                            tt(dst, tA, dst, op) if f else \
                                tt(dst, dst, tA, op)

            scr_d = (r0d_t, r1d_t, tAd_t)
            scr_p = (r0p_t, r1p_t, tAp_t, tBp_t)

            # Pool: independent pipelined chunks over [EB, E)
            for c0, c1 in zip(POOL_CHUNKS[:-1], POOL_CHUNKS[1:]):
                if c1 > c0:
                    emit_bc(nc.gpsimd, c0, c1, locp_t[c0], c0, locp_w[c0],
                            scr_p, EB, EPW)

            # DVE: one full sweep [0, EB) — fewer, wider instructions
            # (DVE is serial anyway; splitting only helped Pool overlap)
            emit_bc(nc.vector, 0, EB, locd_t, 0, ED, scr_d, 0, ED)

            # ---------------- tree helpers ----------------
            def emit_rep(eng_copy, dst_tile, src_tile, src_m, src_n0, q,
                         src_is_loc=False):
                # dst[(s,i,l, 2q block)] = src[(s,i,l, src_n0 + (n>>1))]
                m = 2 * q
                if src_is_loc:
                    src = ap(src_tile, src_n0,
                             [[4 * ED, NSUB * 3], [ED, 4], [1, q], [0, 2]])
                else:
                    src = ap(src_tile, src_n0,
                             [[4 * src_m, NSUB * 3], [src_m, 4],
                              [1, q], [0, 2]])
                dst = ap(dst_tile, 0,
                         [[4 * m, NSUB * 3], [m, 4], [1, m]])
                eng_copy(dst, src)

            def loc_lookup(eg, cnt):
                # which locals tile holds global edges [eg, eg+cnt)
                if eg < EB:
                    assert eg + cnt <= EB
                    return locd_t, ED, eg
                for c0, c1 in zip(POOL_CHUNKS[:-1], POOL_CHUNKS[1:]):
                    if c0 <= eg < c1:
                        assert eg + cnt <= c1
                        return locp_t[c0], locp_w[c0], eg - c0
                raise AssertionError(eg)

            def emit_level_chunk(eng, wt, wm, wn0, rep_tile, rm, rn0,
                                 eg, cnt):
                # wt: per-chunk tile (s,i,l,cnt); rep_tile: replicated
                # parents (s,i,l,rm) at [rn0, rn0+cnt); eg: global edge of
                # the chunk's first node.  DVE TensorTensor is limited to
                # 3 free dims, so the k-muls are split per output row i;
                # the k-accumulation adds run on the fully-merged flat tile.
                assert wm == cnt and wn0 == 0
                tt = eng.tensor_tensor
                ltile, lw, lo_ = loc_lookup(eg, cnt)
                c = cnt
                for k in range(3):
                    for i in range(3):
                        repki = ap(rep_tile, i * 4 * rm + k * rm + rn0,
                                   [[12 * rm, NSUB], [0, 4], [1, c]])
                        lk = ap(ltile, k * 4 * lw + lo_,
                                [[12 * lw, NSUB], [lw, 4], [1, c]])
                        base = wt if k == 0 else tmp_t
                        dst = ap(base, i * 4 * c,
                                 [[12 * c, NSUB], [c, 4], [1, c]])
                        tt(dst, repki, lk, Alu.mult)
                    if k > 0:
                        flat_w = ap(wt, 0, [[1, NSUB * 12 * c]])
                        flat_t = ap(tmp_t, 0, [[1, NSUB * 12 * c]])
                        tt(flat_w, flat_w, flat_t, Alu.add)
                wtr = ap(wt, 3 * c, [[4 * c, NSUB * 3], [1, c]])
                rtr = ap(rep_tile, 3 * rm + rn0,
                         [[4 * rm, NSUB * 3], [1, c]])
                tt(wtr, wtr, rtr, Alu.add)

            shuf = nc.scalar.copy

            def emit_shuffle(src_tile, m, lo, n0, n1, eng_copy=None):
                cnt = n1 - n0
                dst = out_ap(lo + n0, cnt, 0, [[16, cnt], [4, 3], [1, 4]])
                src = ap(src_tile, n0,
                         [[12 * m, NSUB], [1, cnt], [4 * m, 3], [m, 4]])
                (eng_copy or shuf)(dst, src)

            out_v = out_d[:].rearrange("(s p) m -> p s m", p=P)

            def emit_dma(nlo, nhi):
                ot = out_ts[nlo]
                cw = 16 * (nhi - nlo)
                for s in range(NSUB):
                    nc.sync.dma_start(
                        out_v[:, s, nlo * 16:nhi * 16],
                        ot[:, s * cw:(s + 1) * cw])

            # ---- small levels d2..d5: all-DVE (rep via DVE copy) ----
            # high_priority: tree ops dispatch ahead of BC-B whenever ready,
            # so downstream consumers' engine-counter waits don't inflate.
            with tc.high_priority():
                for i, (lo, hi) in enumerate(SMALL_LEVELS):
                    m = hi - lo
                    rep_tile = repa_t if i % 2 == 0 else repb_t
                    if lo == 3:
                        emit_rep(nc.vector.tensor_copy, rep_tile, locd_t,
                                 None, 0, m // 2, src_is_loc=True)
                    else:
                        emit_rep(nc.vector.tensor_copy, rep_tile, w_t[plo],
                                 pm, 0, m // 2)
                    emit_level_chunk(nc.vector, w_t[lo], m, 0, rep_tile,
                                     m, 0, lo - 1, m)
                    plo, pm = lo, m

            # ACT: shuffle nodes 1-2 (locals e0,e1) + small levels
            dst12 = out_ap(1, 2, 0, [[16, 2], [4, 3], [1, 4]])
            src12 = ap(locd_t, 0,
                       [[12 * ED, NSUB], [1, 2], [4 * ED, 3], [ED, 4]])
            shuf(dst12, src12)
            for (lo, hi) in SMALL_LEVELS:
                emit_shuffle(w_t[lo], hi - lo, lo, 0, hi - lo)
            emit_dma(0, 31)
            emit_dma(31, 63)

            # ---- d6: rep + compute on DVE (high priority) ----
            lo6, hi6 = D6
            m6 = hi6 - lo6
            lo7, hi7 = D7
            m7 = hi7 - lo7
            e7 = lo7 - 1                       # first edge of d7 = 126
            with tc.high_priority():
                emit_rep(nc.vector.tensor_copy, repa_t, w_t[31], 32, 0, 32)
                emit_level_chunk(nc.vector, w_t[lo6], m6, 0, repa_t, m6, 0,
                                 lo6 - 1, m6)
                # d7 rep on DVE in-stream (a cross-engine ACT rep
                # stalls DVE ~5.6us at the d6->d7 transition)
                emit_rep(nc.vector.tensor_copy, repb_t, w_t[lo6], m6, 0, m6)
            emit_shuffle(w_t[lo6], m6, lo6, 0, m6)
            emit_dma(63, 127)

            # ---- d7: compute on DVE in edge-aligned chunks ----
            for n0, n1 in zip(d7_bounds[:-1], d7_bounds[1:]):
                cnt = n1 - n0
                if cnt == 0:
                    continue
                with tc.high_priority():
                    emit_level_chunk(nc.vector, w7c_t[n0], cnt, 0, repb_t,
                                     m7, n0, e7 + n0, cnt)
                emit_shuffle(w7c_t[n0], cnt, lo7 + n0, 0, cnt)
                if n1 < m7:
                    emit_dma(lo7 + n0, lo7 + n1)

            # ---- node 255 = child of node 127 (n-index 0 of d7 chunk 0);
            # computed on Pool: it reads Pool's own last locals chunk and
            # Pool is idle by then (its BC is done) ----
            w7c0 = w7c_t[0]
            c1w = d7_bounds[1] - d7_bounds[0]
            l255, lw255, lo255 = loc_lookup(254, 1)
            tmp8_t = tmp_p8
            for k in range(3):
                repk = ap(w7c0, k * c1w,
                          [[12 * c1w, NSUB], [4 * c1w, 3], [0, 4]])
                lk = ap(l255, k * 4 * lw255 + lo255,
                        [[12 * lw255, NSUB], [0, 3], [lw255, 4]])
                dstk = ap(w8_t, 0, [[12, NSUB], [4, 3], [1, 4]])
                if k == 0:
                    nc.gpsimd.tensor_tensor(dstk, repk, lk, Alu.mult)
                else:
                    tmpk = ap(tmp8_t, 0, [[12, NSUB], [4, 3], [1, 4]])
                    nc.gpsimd.tensor_tensor(tmpk, repk, lk, Alu.mult)
                    nc.gpsimd.tensor_tensor(dstk, dstk, tmpk, Alu.add)
            w8tr = ap(w8_t, 3, [[12, NSUB], [4, 3]])
            w7tr = ap(w7c0, 3 * c1w, [[12 * c1w, NSUB], [4 * c1w, 3]])
            nc.gpsimd.tensor_tensor(w8tr, w8tr, w7tr, Alu.add)
            dst255 = out_ap(255, 1, 0, [[4, 3], [1, 4]])
            src255 = ap(w8_t, 0, [[12, NSUB], [4, 3], [1, 4]])
            shuf(dst255, src255)
            emit_dma(lo7 + d7_bounds[-2], 256)

    nc.compile()
    return nc


# --------------------------------------------------------------------------- #
# cached PJRT runner (axon path) — compile once, execute per call
# --------------------------------------------------------------------------- #
def _get_runner(general_constraints, sc_const, of_const, loop_n=1):
    key = ("runner", general_constraints, round(sc_const, 6),
           round(of_const, 6), loop_n)
    if key in _state:
        return _state[key]

    import jax
    from jax.sharding import Mesh, PartitionSpec, NamedSharding
    from jax.experimental.shard_map import shard_map
    import concourse.mybir as mybir
    from concourse import bass2jax

    nc = _build_nc(general_constraints, sc_const, of_const, loop_n)
    bass2jax.install_neuronx_cc_hook()

    part_name = (nc.partition_id_tensor.name
                 if nc.partition_id_tensor is not None else None)
    in_names, out_names, out_avals = [], [], []
    for alloc in nc.m.functions[0].allocations:
        if not isinstance(alloc, mybir.MemoryLocationSet):
            continue
        name = alloc.memorylocations[0].name
        if alloc.kind == "ExternalInput":
            if name != part_name:
                in_names.append(name)
        elif alloc.kind == "ExternalOutput":
            out_names.append(name)
            out_avals.append(jax.core.ShapedArray(
                tuple(alloc.tensor_shape), mybir.dt.np(alloc.dtype)))
    n_params = len(in_names)
    all_in = in_names + out_names
    if part_name is not None:
        all_in = all_in + [part_name]

    def _body(*args):
        operands = list(args)
        if part_name is not None:
            operands.append(bass2jax.partition_id_tensor())
        outs = bass2jax._bass_exec_p.bind(
            *operands,
            out_avals=tuple(out_avals),
            in_names=tuple(all_in),
            out_names=tuple(out_names),
            lowering_input_output_aliases=(),
            sim_require_finite=True,
            sim_require_nnan=True,
            nc=nc,
        )
        return tuple(outs)

    devices = jax.devices()[:NCORE]
    mesh = Mesh(np.asarray(devices), ("core",))
    nin = n_params + len(out_names)
    sharded = jax.jit(
        shard_map(_body, mesh=mesh,
                  in_specs=(PartitionSpec("core"),) * nin,
                  out_specs=(PartitionSpec("core"),) * len(out_names),
                  check_rep=False),
        donate_argnums=tuple(range(n_params, nin)),
        keep_unused=True,
    )
    shard0 = NamedSharding(mesh, PartitionSpec("core"))

    def _make_zeros():
        return jax.jit(
            lambda: jax.numpy.zeros((NCORE * BPC, N * 16), np.float32),
            out_shardings=shard0)()

    runner = (sharded, in_names, _make_zeros)
    _state[key] = runner
    return runner


def make_feed(log_angles, tip_to_base):
    """Build the device feed dict: la (f32) and tip as (k, l, e-pad256) fp16."""
    tip_kle = np.zeros((3, 4, 256), np.float16)
    tip_kle[:, :, :E] = np.transpose(
        tip_to_base[:, :3, :], (1, 2, 0)).astype(np.float16)
    tip_kle = tip_kle.reshape(1, 12 * 256)
    return {
        "la": np.ascontiguousarray(log_angles, dtype=np.float32),
        "tip": np.broadcast_to(tip_kle, (NCORE, 12 * 256)).copy(),
    }


def _run_device(log_angles, tip_to_base, sc_const, of_const, loop_n=1):
    sharded, in_names, make_zeros = _get_runner(
        False, sc_const, of_const, loop_n)
    feed = make_feed(log_angles, tip_to_base)
    args = [feed[name] for name in in_names]
    out = sharded(*args, make_zeros())[0]
    return np.asarray(out).reshape(B, N, 4, 4)


# --------------------------------------------------------------------------- #
# public entry point
# --------------------------------------------------------------------------- #
def kernel(log_angles, tip_to_base, rot_axes, rot_constraints):
    log_angles = np.asarray(log_angles)
    tip_to_base = np.asarray(tip_to_base)
    rot_axes = np.asarray(rot_axes)
    rot_constraints = np.asarray(rot_constraints)

    expected_shapes = (log_angles.shape == (B, J)
                       and tip_to_base.shape == (E, 4, 4)
                       and rot_axes.shape == (J, 3)
                       and rot_constraints.shape == (J, 2))
    eye_tiled = np.tile(np.eye(3, dtype=np.float32), (E, 1)) \
        if expected_shapes else None
    euler = expected_shapes and np.allclose(rot_axes, eye_tiled, atol=1e-6)
    if not euler:
        return _np_fallback(log_angles, tip_to_base, rot_axes, rot_constraints)

    sc = rot_constraints[:, 0].astype(np.float32)
    of = rot_constraints[:, 1].astype(np.float32)
    const_ok = (np.all(sc == sc[0]) and np.all(of == of[0])
                and float(of[0]) == 0.0
                and float(sc[0]) > 1e-3
                and abs(float(sc[0])) + abs(float(of[0])) <= PI + 1e-4)
    if not const_ok:
        # untested-on-device parameter regime: use the exact host fallback
        return _np_fallback(log_angles, tip_to_base, rot_axes,
                            rot_constraints)

    out = _run_device(log_angles, tip_to_base, float(sc[0]), float(of[0]))
    return out


# revision 49
# speedup vs baseline: 1.1486x; 1.1486x over previous
"""Trainium2 Bass kernel for nn_BodyKinematics (batched tree forward kinematics).

Contract: kernel(**inputs) takes the FULL unsharded inputs as numpy arrays and
returns the FULL output (B, N, 4, 4) float32.  Internally the batch dim is
sharded across 8 NeuronCores (pure data parallelism); the tiny per-edge
parameters are replicated.

Math (matches the jax reference):
  theta = tanh(log_angles) * scale                     # (B, 3E), offset == 0
  per edge e: r = Rx(th_x) @ Ry(th_y) @ Rz(th_z)       # axes are e_x, e_y, e_z
  local_e  = [r | 0; 0 1] @ tip_to_base[e]             # affine 3x4 is enough
  tree: W_0 = I, W_n = W_parent(n) @ local_{n-1}       # parent(n) = (n-1)//2
  output: W as 4x4 with constant bottom row (0,0,0,1)

Device design (per core, 512 batch rows = 4 subtiles x 128 partitions):
  partitions = batch-within-subtile; free dim = per-edge structure with the
  edge/node index INNERMOST; fp16 compute everywhere so every TensorTensor
  hits the DVE 2x_1p fast mode (2-byte packed operands).  All 4 subtiles are
  merged into single wide instructions.

  Engine plan: DVE does BC edges [0,EB) plus the whole tree; GpSimd (slow at
  TensorTensor) independently does BC edges [EB,E) in pipelined e-chunks so
  its locals arrive progressively for the deep tree level; ACT does
  tanh/sin/|t|/cos, the parent->children replication copies for big tree
  levels, and the fp16->fp32 shuffle into the node-major fp32 output tile.
  Output DMAs are issued per node-range as soon as their shuffle lands.

  A single activation table (silu_and_others: tanh+sin+abs+copy) serves all
  ACT ops so no per-iteration table reloads happen.
"""

import os
import sys

for _p in ("/opt/trn_rl_repo",):
    if _p not in sys.path and os.path.isdir(_p):
        sys.path.insert(0, _p)

import numpy as np

B, E, N = 4096, 255, 256
J = 3 * E           # 765 angles
NCORE, P, NSUB = 8, 128, 4
BPC = P * NSUB      # 512 batch rows per core
PI = float(np.pi)

# BC edge-range split (tuning knobs)
EA = 62                           # chunk A: all-DVE, feeds small tree levels
EB = 250                          # DVE does [EA, EB); Pool does [EB, E)
POOL_CHUNKS = [250, 255]            # Pool pipeline chunk boundaries
# NOTE: GpSimd fp16 TensorTensor is ~1.7x slower on HW than the cost model,
# and concurrent Pool BC interferes with DVE non-monotonically (measured:
# Pool-95-edges 129us, Pool-67 146us, Pool-5 122us).  Best measured config
# keeps BC almost entirely on DVE.
JP = 3 * 256                      # axis-padded angle pitch (4B alignment)

_state: dict = {}


# --------------------------------------------------------------------------- #
# numpy fallback (exact float32 port of the reference) — used only if the
# inputs don't match the structure the device kernel was built for.
# --------------------------------------------------------------------------- #
def _np_skew(a):
    x, y, z = a[..., 0], a[..., 1], a[..., 2]
    zero = np.zeros_like(x)
    return np.stack([
        np.stack([zero, -z, y], -1),
        np.stack([z, zero, -x], -1),
        np.stack([-y, x, zero], -1)], -2)


def _np_fallback(log_angles, tip_to_base, rot_axes, rot_constraints):
    la = log_angles.astype(np.float32)
    b, e3 = la.shape
    e = e3 // 3
    n = e + 1
    theta = np.tanh(la) * rot_constraints[:, 0] + rot_constraints[:, 1]
    K = _np_skew(rot_axes.astype(np.float32))
    K2 = np.einsum('mij,mjk->mik', K, K).astype(np.float32)
    s = np.sin(theta)[..., None, None]
    c = (1.0 - np.cos(theta))[..., None, None]
    I3 = np.eye(3, dtype=np.float32)
    rots = (I3 + s * K + c * K2).reshape(b, e, 3, 3, 3).astype(np.float32)
    r = np.einsum('beij,bejk,bekl->beil', rots[:, :, 0], rots[:, :, 1],
                  rots[:, :, 2]).astype(np.float32)
    T = np.zeros((b, e, 4, 4), np.float32)
    T[..., :3, :3] = r
    T[..., 3, 3] = 1.0
    local = np.einsum('beij,ejk->beik', T,
                      tip_to_base.astype(np.float32)).astype(np.float32)
    worlds = np.zeros((b, n, 4, 4), np.float32)
    worlds[:, 0] = np.eye(4, dtype=np.float32)
    for i in range(1, n):
        par = (i - 1) // 2
        worlds[:, i] = (worlds[:, par] @ local[:, i - 1]).astype(np.float32)
    return worlds


# tree levels: node range [lo, hi), m = hi - lo nodes, edges [lo-1, hi-1)
SMALL_LEVELS = [(3, 7), (7, 15), (15, 31), (31, 63)]
D6 = (63, 127)
D7 = (127, 255)


def _patch_act_tables():
    """Restrict activation-table selection to silu_and_others (which holds
    tanh, sin, abs, copy) so a single LoadActFuncSet, hoisted out of the
    loop, serves every ACT op."""
    import concourse.hw_specs as hw_specs
    import concourse.bacc as bacc
    if getattr(bacc, "_act_tables_patched", False):
        return
    orig = hw_specs.get_activation_tables

    def patched(arch):
        tabs = orig(arch)
        keep = None
        for name, funcs in tabs.items():
            import concourse.mybir as mybir
            AFT = mybir.ActivationFunctionType
            if (AFT.Tanh in funcs and AFT.Sin in funcs and AFT.Abs in funcs
                    and AFT.Copy in funcs):
                keep = name
                break
        if keep is None:
            return tabs
        return {name: (funcs if name == keep else frozenset())
                for name, funcs in tabs.items()}

    bacc.get_activation_tables = patched
    bacc._act_tables_patched = True


# --------------------------------------------------------------------------- #
# device kernel build
# --------------------------------------------------------------------------- #
def _build_nc(general_constraints: bool, sc_const: float, of_const: float,
              loop_n: int = 1):
    assert not general_constraints, "fast path requires uniform constraints"
    import concourse.bacc as bacc
    import concourse.mybir as mybir
    from concourse.tile import TileContext
    import concourse.bass as bass
    from contextlib import ExitStack

    f32 = mybir.dt.float32
    f16 = mybir.dt.float16
    i32 = mybir.dt.int32
    Alu = mybir.AluOpType
    AFT = mybir.ActivationFunctionType

    scv = float(sc_const)
    assert float(of_const) == 0.0, "fast path assumes zero offset"

    _patch_act_tables()
    nc = bacc.Bacc("TRN2", target_bir_lowering=False, debug=False)

    la_d = nc.dram_tensor("la", [BPC, J], f32, kind="ExternalInput")
    # tip prepared host-side as (k, l, e-pad256) fp16, flat [1, 12*256]
    tip_d = nc.dram_tensor("tip", [1, 12 * 256], f16, kind="ExternalInput")
    out_d = nc.dram_tensor("out", [BPC, N * 16], f32, kind="ExternalOutput")

    with TileContext(nc) as tc:
        with tc.tile_pool(name="main", bufs=1) as pool, \
             tc.tile_pool(name="ps", bufs=1, space="PSUM") as psp, \
             ExitStack() as _loop_ctx:

            ED = EB              # DVE locals/scratch cover edges [0, EB)
            EP = E - EB          # Pool covers [EB, E)
            # angle tiles use a 256-padded axis pitch so every fp16
            # operand lands 4B-aligned (required for the DVE 2x_1p mode)
            la_t = pool.tile([P, NSUB * JP], f32)   # raw angles; reused as |t|
            th_t = pool.tile([P, NSUB * JP], f32)   # theta, (s, a, e) order
            tip_t = pool.tile([P, 12 * 256], f16)   # (k, l, e-padded)
            # NOTE: tiles shared by two engines must live in SBUF — the tile
            # scheduler serializes ALL cross-engine PSUM accesses (even
            # read-read), which would chain DVE behind every Pool op.
            sin_t = pool.tile([P, NSUB * JP], f16)  # (s, a, e-padded)
            cos_t = pool.tile([P, NSUB * JP], f16)
            # per-engine locals + scratch (separate tiles so the dep tracker
            # never serializes DVE behind Pool); Pool locals are per-chunk
            # tiles so the deep-level reads depend only on their own chunk.
            locd_t = pool.tile([P, NSUB * 12 * ED], f16)   # (s, k, l, e<EB)
            locp_t = {}
            locp_w = {}
            for c0, c1 in zip(POOL_CHUNKS[:-1], POOL_CHUNKS[1:]):
                cw = (c1 - c0 + 1) & ~1          # even-padded width
                locp_w[c0] = cw
                locp_t[c0] = pool.tile([P, NSUB * 12 * cw], f16,
                                       name=f"locp{c0}")
            # DVE scratch in PSUM (DVE-exclusive, so no cross-engine edges);
            # q2 aliases r0 (r0 is dead once the y-stage q2 mul runs).
            r0d_t = pool.tile([P, NSUB * 4 * ED], f16)
            r1d_t = pool.tile([P, NSUB * 4 * ED], f16)
            tAd_t = pool.tile([P, NSUB * 4 * ED], f16)
            EPW = (EP + 1) & ~1              # even-padded Pool width
            r0p_t = pool.tile([P, NSUB * 4 * EPW], f16)
            r1p_t = pool.tile([P, NSUB * 4 * EPW], f16)
            tAp_t = pool.tile([P, NSUB * 4 * EPW], f16)
            tBp_t = pool.tile([P, NSUB * 4 * EPW], f16)
            w_t = {}
            for (lo, hi) in SMALL_LEVELS + [D6]:
                m = hi - lo
                w_t[lo] = pool.tile([P, NSUB * 12 * m], f16, name=f"w{lo}")
            d7_bounds = sorted(set(
                [0, min(EB - 126, 128)] +
                [min(c - 126, 128) for c in POOL_CHUNKS[1:]]))
            i_ = 0                      # subdivide any chunk wider than 64
            while i_ < len(d7_bounds) - 1:
                a_, b_ = d7_bounds[i_], d7_bounds[i_ + 1]
                if b_ - a_ > 64:
                    d7_bounds.insert(i_ + 1, a_ + (b_ - a_ + 1) // 2 // 2 * 2)
                else:
                    i_ += 1
            w7c_t = {}
            for n0, n1 in zip(d7_bounds[:-1], d7_bounds[1:]):
                w7c_t[n0] = pool.tile([P, NSUB * 12 * (n1 - n0)], f16,
                                      name=f"w7c{n0}")
            w8_t = pool.tile([P, NSUB * 12], f16)        # node 255
            tmp_p8 = pool.tile([P, NSUB * 12], f16)      # node-255 scratch
            tmp_t = pool.tile([P, NSUB * 12 * 64], f16)  # chunk mul scratch
            repa_t = pool.tile([P, NSUB * 12 * 64], f16)
            repb_t = pool.tile([P, NSUB * 12 * 128], f16)
            # per-DMA-chunk output tiles (separate so a shuffle never WARs
            # against the previous chunk's DMA read)
            out_chunks = [(0, 31), (31, 63), (63, 127)] + \
                [(127 + a, 127 + b) for a, b in
                 zip(d7_bounds[:-2], d7_bounds[1:-1])] + \
                [(127 + d7_bounds[-2], 256)]
            out_ts = {}
            for (nlo, nhi) in out_chunks:
                out_ts[nlo] = pool.tile([P, NSUB * (nhi - nlo) * 16], f32,
                                        name=f"out{nlo}")
            hpi_t = pool.tile([P, 1], f32)

            def out_ap(node0, cnt, sub_off, dims_tail):
                # AP into the out-chunk tile holding [node0, node0+cnt):
                # dims: [partition] + [[chunk-sub-stride, NSUB]] + dims_tail
                for (nlo, nhi) in out_chunks:
                    if nlo <= node0 and node0 + cnt <= nhi:
                        cw = (nhi - nlo) * 16
                        return ap(out_ts[nlo],
                                  (node0 - nlo) * 16 + sub_off,
                                  [[cw, NSUB]] + dims_tail)
                raise AssertionError((node0, cnt))

            def ap(tile, off, dims):
                a0 = tile[:]
                return bass.AP(a0.tensor, a0.offset + off,
                               [list(a0.ap[0])] + [list(d) for d in dims])

            # ---- one-time init: pi/2 bias + constant parts of out tiles ----
            nc.gpsimd.memset(hpi_t[:], PI / 2.0)
            for (nlo, nhi) in out_chunks:
                cn = nhi - nlo
                ot = out_ts[nlo]
                nc.gpsimd.memset(ap(ot, 12, [[16, NSUB * cn], [1, 3]]), 0.0)
                nc.gpsimd.memset(ap(ot, 15, [[16, NSUB * cn]]), 1.0)
            for s in range(NSUB):
                nc.gpsimd.memset(ap(out_ts[0], s * 31 * 16,
                                    [[4, 3], [1, 4]]), 0.0)
                nc.gpsimd.memset(ap(out_ts[0], s * 31 * 16, [[5, 3]]), 1.0)

            if loop_n > 1:
                _loop_ctx.enter_context(tc.For_i(0, loop_n, 1))

            # ---------------- input DMAs ----------------
            la_v = la_d[:].rearrange("(s p) j -> p s j", p=P)   # [128, 4, 765]
            for s in range(NSUB):
                nc.sync.dma_start(la_t[:, s * JP:s * JP + J], la_v[:, s])
            nc.sync.dma_start(tip_t[:],
                              bass.AP(tip_d, 0, [[0, P], [1, 12 * 256]]))

            # ---------------- angles: tanh, then per-axis sin/|t|/cos -------
            act = nc.scalar.activation
            # tanh with fused (e,a) -> (a,e) deinterleave, per sub so each
            # starts as soon as its input DMA lands
            for s in range(NSUB):
                act(ap(th_t, s * JP, [[256, 3], [1, 255]]),
                    ap(la_t, s * JP, [[1, 3], [3, 255]]),
                    AFT.Tanh)
            # per axis (z first so BC stage z can start earliest):
            #   sin_a = Sin(scv * th_a); ab_a = |th_a|  (into la_t, (a,e));
            #   cos_a = Sin(pi/2 - scv*ab_a)
            for a_ax in (2, 1, 0):
                tha = ap(th_t, a_ax * 256, [[JP, NSUB], [1, 255]])
                sina = ap(sin_t, a_ax * 256, [[JP, NSUB], [1, 255]])
                aba = ap(la_t, a_ax * 256, [[JP, NSUB], [1, 255]])
                cosa = ap(cos_t, a_ax * 256, [[JP, NSUB], [1, 255]])
                act(sina, tha, AFT.Sin, scale=scv)
                act(aba, tha, AFT.Abs)
                act(cosa, aba, AFT.Sin, bias=hpi_t[:], scale=-scv)

            # ---------------- BC: locals = Rx Ry Rz @ tip ----------------
            #   z: r0 = cz*T0 - sz*T1 ; r1 = sz*T0 + cz*T1
            #   y: L0 = cy*r0 + sy*T2 ; q2 = cy*T2 - sy*r0
            #   x: L1 = cx*r1 - sx*q2 ; L2 = sx*r1 + cx*q2
            def emit_bc(eng, e0, e1, loc_tile, lbase, lw, scr, sbase, sw):
                w = e1 - e0
                lo_ = e0 - lbase
                so = e0 - sbase

                def trig(t, a_ax):
                    return ap(t, a_ax * 256 + e0,
                              [[JP, NSUB], [0, 4], [1, w]])

                def tipr(k):
                    return ap(tip_t, k * 4 * 256 + e0,
                              [[0, NSUB], [256, 4], [1, w]])

                def sle(t):
                    return ap(t, so, [[4 * sw, NSUB], [sw, 4], [1, w]])

                def locr(k):
                    return ap(loc_tile, k * 4 * lw + lo_,
                              [[12 * lw, NSUB], [lw, 4], [1, w]])

                sx, sy, sz = (trig(sin_t, a) for a in range(3))
                cx, cy, cz = (trig(cos_t, a) for a in range(3))
                T0, T1, T2 = (tipr(k) for k in range(3))
                L0, L1, L2 = (locr(k) for k in range(3))
                r0, r1, tA = (sle(t) for t in scr[:3])
                tB = sle(scr[3]) if len(scr) > 3 else None
                q2 = r0        # alias: r0 dead after the q2 in-place mul
                stages = [
                    [(cz, T0, sz, T1, r0, Alu.subtract, True),
                     (sz, T0, cz, T1, r1, Alu.add, False)],
                    [(cy, r0, sy, T2, L0, Alu.add, False),
                     (cy, T2, sy, r0, q2, Alu.subtract, True)],
                    [(cx, r1, sx, q2, L1, Alu.subtract, True),
                     (sx, r1, cx, q2, L2, Alu.add, False)],
                ]
                tt = eng.tensor_tensor
                if tB is not None:
                    # per stage: both triples' muls first (two tA buffers),
                    # then the combines — keeps a 2-wide ready window so
                    # Pool never idles into a cross-chunk slip-in.
                    for (t1, t2) in stages:
                        (a1, b1, c1, d1, dst1, op1, f1) = t1
                        (a2, b2, c2, d2, dst2, op2, f2) = t2
                        tt(tA, a1, b1, Alu.mult)
                        tt(dst1, c1, d1, Alu.mult)
                        tt(tB, a2, b2, Alu.mult)
                        tt(dst2, c2, d2, Alu.mult)
                        tt(dst1, tA, dst1, op1) if f1 else \
                            tt(dst1, dst1, tA, op1)
                        tt(dst2, tB, dst2, op2) if f2 else \
                            tt(dst2, dst2, tB, op2)
                else:
                    for (t1, t2) in stages:
                        for (a, b, c, d, dst, op, f) in (t1, t2):
                            tt(tA, a, b, Alu.mult)
                            tt(dst, c, d, Alu.mult)
                            tt(dst, tA, dst, op) if f else \
                                tt(dst, dst, tA, op)

            scr_d = (r0d_t, r1d_t, tAd_t)
            scr_p = (r0p_t, r1p_t, tAp_t, tBp_t)

            # Pool: independent pipelined chunks over [EB, E)
            for c0, c1 in zip(POOL_CHUNKS[:-1], POOL_CHUNKS[1:]):
                if c1 > c0:
                    emit_bc(nc.gpsimd, c0, c1, locp_t[c0], c0, locp_w[c0],
                            scr_p, EB, EPW)

            # DVE chunk A (feeds small tree), then the rest up to EB
            emit_bc(nc.vector, 0, EA, locd_t, 0, ED, scr_d, 0, ED)

            # ---------------- tree helpers ----------------
            def emit_rep(eng_copy, dst_tile, src_tile, src_m, src_n0, q,
                         src_is_loc=False):
                # dst[(s,i,l, 2q block)] = src[(s,i,l, src_n0 + (n>>1))]
                m = 2 * q
                if src_is_loc:
                    src = ap(src_tile, src_n0,
                             [[4 * ED, NSUB * 3], [ED, 4], [1, q], [0, 2]])
                else:
                    src = ap(src_tile, src_n0,
                             [[4 * src_m, NSUB * 3], [src_m, 4],
                              [1, q], [0, 2]])
                dst = ap(dst_tile, 0,
                         [[4 * m, NSUB * 3], [m, 4], [1, m]])
                eng_copy(dst, src)

            def loc_lookup(eg, cnt):
                # which locals tile holds global edges [eg, eg+cnt)
                if eg < EB:
                    assert eg + cnt <= EB
                    return locd_t, ED, eg
                for c0, c1 in zip(POOL_CHUNKS[:-1], POOL_CHUNKS[1:]):
                    if c0 <= eg < c1:
                        assert eg + cnt <= c1
                        return locp_t[c0], locp_w[c0], eg - c0
                raise AssertionError(eg)

            def emit_level_chunk(eng, wt, wm, wn0, rep_tile, rm, rn0,
                                 eg, cnt):
                # wt: per-chunk tile (s,i,l,cnt); rep_tile: replicated
                # parents (s,i,l,rm) at [rn0, rn0+cnt); eg: global edge of
                # the chunk's first node.  DVE TensorTensor is limited to
                # 3 free dims, so the k-muls are split per output row i;
                # the k-accumulation adds run on the fully-merged flat tile.
                assert wm == cnt and wn0 == 0
                tt = eng.tensor_tensor
                ltile, lw, lo_ = loc_lookup(eg, cnt)
                c = cnt
                for k in range(3):
                    for i in range(3):
                        repki = ap(rep_tile, i * 4 * rm + k * rm + rn0,
                                   [[12 * rm, NSUB], [0, 4], [1, c]])
                        lk = ap(ltile, k * 4 * lw + lo_,
                                [[12 * lw, NSUB], [lw, 4], [1, c]])
                        base = wt if k == 0 else tmp_t
                        dst = ap(base, i * 4 * c,
                                 [[12 * c, NSUB], [c, 4], [1, c]])
                        tt(dst, repki, lk, Alu.mult)
                    if k > 0:
                        flat_w = ap(wt, 0, [[1, NSUB * 12 * c]])
                        flat_t = ap(tmp_t, 0, [[1, NSUB * 12 * c]])
                        tt(flat_w, flat_w, flat_t, Alu.add)
                wtr = ap(wt, 3 * c, [[4 * c, NSUB * 3], [1, c]])
                rtr = ap(rep_tile, 3 * rm + rn0,
                         [[4 * rm, NSUB * 3], [1, c]])
                tt(wtr, wtr, rtr, Alu.add)

            shuf = nc.scalar.copy

            def emit_shuffle(src_tile, m, lo, n0, n1, eng_copy=None):
                cnt = n1 - n0
                dst = out_ap(lo + n0, cnt, 0, [[16, cnt], [4, 3], [1, 4]])
                src = ap(src_tile, n0,
                         [[12 * m, NSUB], [1, cnt], [4 * m, 3], [m, 4]])
                (eng_copy or shuf)(dst, src)

            out_v = out_d[:].rearrange("(s p) m -> p s m", p=P)

            def emit_dma(nlo, nhi):
                # split each chunk's 4 sub-DMAs across BOTH hardware DMA
                # queues (SP + ACT are the two HWDGE engines) so the output
                # stream drains in parallel instead of serializing on SP
                ot = out_ts[nlo]
                cw = 16 * (nhi - nlo)
                for s in range(NSUB):
                    eng = nc.sync if s % 2 == 0 else nc.scalar
                    eng.dma_start(
                        out_v[:, s, nlo * 16:nhi * 16],
                        ot[:, s * cw:(s + 1) * cw])

            # ---- small levels d2..d5: all-DVE (rep via DVE copy) ----
            # high_priority: tree ops dispatch ahead of BC-B whenever ready,
            # so downstream consumers' engine-counter waits don't inflate.
            with tc.high_priority():
                for i, (lo, hi) in enumerate(SMALL_LEVELS):
                    m = hi - lo
                    rep_tile = repa_t if i % 2 == 0 else repb_t
                    if lo == 3:
                        emit_rep(nc.vector.tensor_copy, rep_tile, locd_t,
                                 None, 0, m // 2, src_is_loc=True)
                    else:
                        emit_rep(nc.vector.tensor_copy, rep_tile, w_t[plo],
                                 pm, 0, m // 2)
                    emit_level_chunk(nc.vector, w_t[lo], m, 0, rep_tile,
                                     m, 0, lo - 1, m)
                    plo, pm = lo, m

            # DVE continues BC chunk B while ACT shuffles small levels
            emit_bc(nc.vector, EA, EB, locd_t, 0, ED, scr_d, 0, ED)

            # ACT: shuffle nodes 1-2 (locals e0,e1) + small levels
            dst12 = out_ap(1, 2, 0, [[16, 2], [4, 3], [1, 4]])
            src12 = ap(locd_t, 0,
                       [[12 * ED, NSUB], [1, 2], [4 * ED, 3], [ED, 4]])
            shuf(dst12, src12)
            for (lo, hi) in SMALL_LEVELS:
                emit_shuffle(w_t[lo], hi - lo, lo, 0, hi - lo)
            emit_dma(0, 31)
            emit_dma(31, 63)

            # ---- d6: rep + compute on DVE (high priority) ----
            lo6, hi6 = D6
            m6 = hi6 - lo6
            lo7, hi7 = D7
            m7 = hi7 - lo7
            e7 = lo7 - 1                       # first edge of d7 = 126
            with tc.high_priority():
                emit_rep(nc.vector.tensor_copy, repa_t, w_t[31], 32, 0, 32)
                emit_level_chunk(nc.vector, w_t[lo6], m6, 0, repa_t, m6, 0,
                                 lo6 - 1, m6)
                # d7 rep on ACT (overlaps DVE's BC-B tail)
                emit_rep(shuf, repb_t, w_t[lo6], m6, 0, m6)
            emit_shuffle(w_t[lo6], m6, lo6, 0, m6)
            emit_dma(63, 127)

            # ---- d7: compute on DVE in edge-aligned chunks ----
            for n0, n1 in zip(d7_bounds[:-1], d7_bounds[1:]):
                cnt = n1 - n0
                if cnt == 0:
                    continue
                with tc.high_priority():
                    emit_level_chunk(nc.vector, w7c_t[n0], cnt, 0, repb_t,
                                     m7, n0, e7 + n0, cnt)
                emit_shuffle(w7c_t[n0], cnt, lo7 + n0, 0, cnt)
                if n1 < m7:
                    emit_dma(lo7 + n0, lo7 + n1)

            # ---- node 255 = child of node 127 (n-index 0 of d7 chunk 0);
            # computed on Pool: it reads Pool's own last locals chunk and
            # Pool is idle by then (its BC is done) ----
            w7c0 = w7c_t[0]
            c1w = d7_bounds[1] - d7_bounds[0]
            l255, lw255, lo255 = loc_lookup(254, 1)
            tmp8_t = tmp_p8
            for k in range(3):
                repk = ap(w7c0, k * c1w,
                          [[12 * c1w, NSUB], [4 * c1w, 3], [0, 4]])
                lk = ap(l255, k * 4 * lw255 + lo255,
                        [[12 * lw255, NSUB], [0, 3], [lw255, 4]])
                dstk = ap(w8_t, 0, [[12, NSUB], [4, 3], [1, 4]])
                if k == 0:
                    nc.gpsimd.tensor_tensor(dstk, repk, lk, Alu.mult)
                else:
                    tmpk = ap(tmp8_t, 0, [[12, NSUB], [4, 3], [1, 4]])
                    nc.gpsimd.tensor_tensor(tmpk, repk, lk, Alu.mult)
                    nc.gpsimd.tensor_tensor(dstk, dstk, tmpk, Alu.add)
            w8tr = ap(w8_t, 3, [[12, NSUB], [4, 3]])
            w7tr = ap(w7c0, 3 * c1w, [[12 * c1w, NSUB], [4 * c1w, 3]])
            nc.gpsimd.tensor_tensor(w8tr, w8tr, w7tr, Alu.add)
            dst255 = out_ap(255, 1, 0, [[4, 3], [1, 4]])
            src255 = ap(w8_t, 0, [[12, NSUB], [4, 3], [1, 4]])
            shuf(dst255, src255)
            emit_dma(lo7 + d7_bounds[-2], 256)

    nc.compile()
    return nc


# --------------------------------------------------------------------------- #
# cached PJRT runner (axon path) — compile once, execute per call
# --------------------------------------------------------------------------- #
def _get_runner(general_constraints, sc_const, of_const, loop_n=1):
    key = ("runner", general_constraints, round(sc_const, 6),
           round(of_const, 6), loop_n)
    if key in _state:
        return _state[key]

    import jax
    from jax.sharding import Mesh, PartitionSpec, NamedSharding
    from jax.experimental.shard_map import shard_map
    import concourse.mybir as mybir
    from concourse import bass2jax

    nc = _build_nc(general_constraints, sc_const, of_const, loop_n)
    bass2jax.install_neuronx_cc_hook()

    part_name = (nc.partition_id_tensor.name
                 if nc.partition_id_tensor is not None else None)
    in_names, out_names, out_avals = [], [], []
    for alloc in nc.m.functions[0].allocations:
        if not isinstance(alloc, mybir.MemoryLocationSet):
            continue
        name = alloc.memorylocations[0].name
        if alloc.kind == "ExternalInput":
            if name != part_name:
                in_names.append(name)
        elif alloc.kind == "ExternalOutput":
            out_names.append(name)
            out_avals.append(jax.core.ShapedArray(
                tuple(alloc.tensor_shape), mybir.dt.np(alloc.dtype)))
    n_params = len(in_names)
    all_in = in_names + out_names
    if part_name is not None:
        all_in = all_in + [part_name]

    def _body(*args):
        operands = list(args)
        if part_name is not None:
            operands.append(bass2jax.partition_id_tensor())
        outs = bass2jax._bass_exec_p.bind(
            *operands,
            out_avals=tuple(out_avals),
            in_names=tuple(all_in),
            out_names=tuple(out_names),
            lowering_input_output_aliases=(),
            sim_require_finite=True,
            sim_require_nnan=True,
            nc=nc,
        )
        return tuple(outs)

    devices = jax.devices()[:NCORE]
    mesh = Mesh(np.asarray(devices), ("core",))
    nin = n_params + len(out_names)
    sharded = jax.jit(
        shard_map(_body, mesh=mesh,
                  in_specs=(PartitionSpec("core"),) * nin,
                  out_specs=(PartitionSpec("core"),) * len(out_names),
                  check_rep=False),
        donate_argnums=tuple(range(n_params, nin)),
        keep_unused=True,
    )
    shard0 = NamedSharding(mesh, PartitionSpec("core"))

    def _make_zeros():
        return jax.jit(
            lambda: jax.numpy.zeros((NCORE * BPC, N * 16), np.float32),
            out_shardings=shard0)()

    runner = (sharded, in_names, _make_zeros)
    _state[key] = runner
    return runner


def make_feed(log_angles, tip_to_base):
    """Build the device feed dict: la (f32) and tip as (k, l, e-pad256) fp16."""
    tip_kle = np.zeros((3, 4, 256), np.float16)
    tip_kle[:, :, :E] = np.transpose(
        tip_to_base[:, :3, :], (1, 2, 0)).astype(np.float16)
    tip_kle = tip_kle.reshape(1, 12 * 256)
    return {
        "la": np.ascontiguousarray(log_angles, dtype=np.float32),
        "tip": np.broadcast_to(tip_kle, (NCORE, 12 * 256)).copy(),
    }


def _run_device(log_angles, tip_to_base, sc_const, of_const, loop_n=1):
    sharded, in_names, make_zeros = _get_runner(
        False, sc_const, of_const, loop_n)
    feed = make_feed(log_angles, tip_to_base)
    args = [feed[name] for name in in_names]
    out = sharded(*args, make_zeros())[0]
    return np.asarray(out).reshape(B, N, 4, 4)


# --------------------------------------------------------------------------- #
# public entry point
# --------------------------------------------------------------------------- #
def kernel(log_angles, tip_to_base, rot_axes, rot_constraints):
    log_angles = np.asarray(log_angles)
    tip_to_base = np.asarray(tip_to_base)
    rot_axes = np.asarray(rot_axes)
    rot_constraints = np.asarray(rot_constraints)

    expected_shapes = (log_angles.shape == (B, J)
                       and tip_to_base.shape == (E, 4, 4)
                       and rot_axes.shape == (J, 3)
                       and rot_constraints.shape == (J, 2))
    eye_tiled = np.tile(np.eye(3, dtype=np.float32), (E, 1)) \
        if expected_shapes else None
    euler = expected_shapes and np.allclose(rot_axes, eye_tiled, atol=1e-6)
    if not euler:
        return _np_fallback(log_angles, tip_to_base, rot_axes, rot_constraints)

    sc = rot_constraints[:, 0].astype(np.float32)
    of = rot_constraints[:, 1].astype(np.float32)
    const_ok = (np.all(sc == sc[0]) and np.all(of == of[0])
                and float(of[0]) == 0.0
                and float(sc[0]) > 1e-3
                and abs(float(sc[0])) + abs(float(of[0])) <= PI + 1e-4)
    if not const_ok:
        # untested-on-device parameter regime: use the exact host fallback
        return _np_fallback(log_angles, tip_to_base, rot_axes,
                            rot_constraints)

    out = _run_device(log_angles, tip_to_base, float(sc[0]), float(of[0]))
    return out
